# revision 39
# baseline (speedup 1.0000x reference)
"""Trainium2 Bass kernel for MixedCausalAttention (16 heads, d=1024, L_S=4096, L_NS=64).

Sharding: tensor-parallel over heads - 2 heads per core x 8 cores.
Each core computes qkv projections for its 2 heads, causal attention, and a
partial W_out product over its 128 output feature rows (head-stacked K=128
matmul). The host sums the 8 partial (2112, 1024) bf16 outputs in f32.

v3 redesign vs v2:
- NS projection fully interleaved into the attention chunk loop as 2-token
  mini-groups (one psS tile = 2 banks, one accumulation group per bank),
  eliminating the serial NS phases that left ACT idle ~100us.
- W_NS descale (x64) folded into the PE transpose identity (x 1/64) and the
  V_NS staging muls - no more per-token single-partition tensor_scalar work.
- qkv_NS staged bf16 via one strided CAST + sbuf->sbuf DMA per mini-group.
- Reciprocal merged to one [2, QT] instr per q-tile (was 2x single-partition).
- V staging: single v_s tile, one CAST per 128-key sub (both heads at once).
- PSUM: psS bufs=3 (6 banks, shared by scores/proj/NS/po/pbc) + av 2 banks.
- DMA order tuned: ws + early x tiles first, wout last, wns streamed with
  bounded prefetch so compute can start ~3us in.
"""

import os
import sys
import math
from concurrent.futures import ThreadPoolExecutor

for _p in ("/opt/trn_rl_repo", "/root/.axon_site/_ro/trn_rl_repo"):
    if os.path.isdir(_p) and _p not in sys.path:
        sys.path.insert(0, _p)

import numpy as np
import ml_dtypes

import concourse.bass as bass
import concourse.mybir as mybir
import concourse.tile as tile
from concourse import bacc
from concourse.bass_utils import run_bass_kernel_spmd

F32 = mybir.dt.float32
BF16 = mybir.dt.bfloat16
F8E4 = mybir.dt.float8e4

N_CORES = 8
D = 1024
H = 16
DH = 64
HPC = H // N_CORES          # heads per core = 2
O3 = 3 * DH * HPC           # 384 qkv output cols per core
LNS = 64
LS = 4096
QS = 2048                   # query_start
LQ = LS - QS + LNS          # 2112 queries
NCH = D // 128              # 8 contraction chunks
ST = 512                    # s-tile width for projections
NT = LS // ST               # 8 s-tiles
QT = 512                    # q-tile width for attention
NG2 = LNS // 2              # 32 NS token mini-groups (2 tokens each)
SCALE = DH ** -0.5
WNS_SCALE = 64.0

n_kc_s = LS // 128          # 32 S key chunks
n_kc = n_kc_s + 1           # + NS chunk
lqs = LS - QS               # 2048 S-query columns


def build_program(repeat=1):
    nc = bacc.Bacc("TRN2", target_bir_lowering=False, debug=False,
                   num_devices=N_CORES)

    xt_d = nc.dram_tensor("xt", [NT, 128, NCH, ST], BF16, kind="ExternalInput")
    xnst_d = nc.dram_tensor("xnst", [128, NCH, LNS], F8E4, kind="ExternalInput")
    ws_d = nc.dram_tensor("ws", [128, NCH, O3], BF16, kind="ExternalInput")
    wns_d = nc.dram_tensor("wns", [NG2, 128, NCH // 2, 2, 2, O3], F8E4,
                           kind="ExternalInput")
    wout_d = nc.dram_tensor("wout", [128, D], BF16, kind="ExternalInput")
    vones_d = nc.dram_tensor("vones", [128, 64], BF16, kind="ExternalInput")
    o_d = nc.dram_tensor("o", [LQ, D], BF16, kind="ExternalOutput")

    with tile.TileContext(nc) as tc:
      for _rep in range(repeat):
        import contextlib
        ctx = contextlib.ExitStack()
        with ctx:
            const = ctx.enter_context(tc.tile_pool(name="const", bufs=1))
            store = ctx.enter_context(tc.tile_pool(name="store", bufs=1))

            # --- constants ---
            # ws first: it gates the very first projection matmul. All the
            # all-ones tiles are memset on gpsimd instead of DMA'd (the
            # strided ones-scatter DMAs took ~3us each to issue on sync).
            ws_sb = const.tile([128, NCH, O3], BF16)
            nc.sync.dma_start(out=ws_sb, in_=ws_d.ap())
            ones_bf = const.tile([128, 64], BF16)
            nc.gpsimd.memset(ones_bf[:, :], 1.0)
            xnst_sb = const.tile([128, NCH, LNS], F8E4)
            nc.sync.dma_start(out=xnst_sb, in_=xnst_d.ap())
            # scaled identity (1/WNS_SCALE on the diagonal) for the NS
            # transposes - folds the W_NS descale into the PE transpose
            ident_sb = const.tile([64, 64], BF16)
            nc.gpsimd.memset(ident_sb[:, :], 0.0)
            nc.gpsimd.affine_select(
                out=ident_sb[:, :], in_=ident_sb[:, :],
                compare_op=mybir.AluOpType.not_equal, fill=1.0 / WNS_SCALE,
                base=0, channel_multiplier=1, pattern=[[-1, 64]])
            wout_sb = const.tile([128, D], BF16)

            # --- persistent activation storage (all bf16) ---
            kt_s = store.tile([128, LS], BF16)      # K^T (h0 rows 0-63, h1 64-127)
            qt_s = store.tile([128, lqs], BF16)     # Q^T, S part
            kt_ns = store.tile([128, LNS], BF16)
            qt_ns = store.tile([128, LNS], BF16)
            # V natural layout: [keys, kc, head, dh]
            v_s = store.tile([128, n_kc_s, HPC, DH], BF16)
            v_ns = store.tile([64, HPC, DH], BF16)
            qkvns_sb = store.tile([LNS, O3], BF16)  # NS qkv rows (x64 scaled)

            xpool = ctx.enter_context(tc.tile_pool(name="xpool", bufs=3))
            wnspool = ctx.enter_context(tc.tile_pool(name="wnspool", bufs=6))
            expool = ctx.enter_context(tc.tile_pool(name="expool", bufs=4))
            dapool = ctx.enter_context(tc.tile_pool(name="dapool", bufs=4))
            rcpool = ctx.enter_context(tc.tile_pool(name="rcpool", bufs=2))
            avtnpool = ctx.enter_context(tc.tile_pool(name="avtnpool", bufs=2))
            outpool = ctx.enter_context(tc.tile_pool(name="outpool", bufs=2))
            stgpool = ctx.enter_context(tc.tile_pool(name="stgpool", bufs=2))
            psS = ctx.enter_context(tc.tile_pool(name="psS", bufs=3, space="PSUM"))
            psAV = ctx.enter_context(tc.tile_pool(name="psAV", bufs=2, space="PSUM"))

            # ---------------- S-token projection units ----------------
            # Emitted lazily; tiles in dependency order [4,0,1,2,3] then
            # [5,6,7] interleaved into the attention chunk loops.
            def sproj_units():
                first = True
                for t in (4, 0, 1, 2, 3, 5, 6, 7):
                    s0 = t * ST
                    xt_t = xpool.tile([128, NCH, ST], BF16, tag="xt", name="xt_t")
                    if first:
                        # split the first tile's load so the first projection
                        # matmul (which reads ci=0 first) starts sooner
                        first = False
                        nc.sync.dma_start(out=xt_t[:, 0:2, :],
                                          in_=xt_d.ap()[t][:, 0:2, :])
                        nc.sync.dma_start(out=xt_t[:, 2:NCH, :],
                                          in_=xt_d.ap()[t][:, 2:NCH, :])
                    else:
                        nc.sync.dma_start(out=xt_t, in_=xt_d.ap()[t])
                    # K^T (and Q^T for t>=4): W_S columns stationary, x moving
                    jobs = [(1, kt_s, s0)]
                    if t >= 4:
                        jobs.append((0, qt_s, s0 - QS))
                    for mi, dest, dcol in jobs:
                        ps = psS.tile([128, 2, QT], F32, tag="psS", name="ps_kq")
                        for ci in range(NCH):
                            nc.tensor.matmul(
                                ps[:, 0, :],
                                lhsT=ws_sb[:, ci, mi * 128:(mi + 1) * 128],
                                rhs=xt_t[:, ci, :],
                                start=(ci == 0), stop=(ci == NCH - 1))
                        nc.vector.tensor_copy(out=dest[:, dcol:dcol + ST],
                                              in_=ps[:, 0, :])
                        yield
                    # V natural: x^T chunk stationary, W_S V-cols moving
                    for sub in range(ST // 128):
                        kc = 4 * t + sub
                        psv = psS.tile([128, 2, QT], F32, tag="psS", name="psv")
                        for ci in range(NCH):
                            nc.tensor.matmul(
                                psv[:, 0, 0:128],
                                lhsT=xt_t[:, ci, sub * 128:(sub + 1) * 128],
                                rhs=ws_sb[:, ci, 256:384],
                                start=(ci == 0), stop=(ci == NCH - 1))
                        # both heads in one strided CAST
                        nc.vector.tensor_copy(
                            out=v_s[:, kc, :, 0:64],
                            in_=psv[:, 0, 0:128])
                        yield

            sproj = sproj_units()
            sproj_left = 8 * 4 + 8 + 4   # 44 units total

            def sproj_step(n=1):
                nonlocal sproj_left
                for _ in range(n):
                    if sproj_left > 0:
                        next(sproj)
                        sproj_left -= 1

            # ---------------- NS-token projection mini-groups ----------------
            wns_tiles = {}
            wns_next_prefetch = 0

            def wns_prefetch():
                nonlocal wns_next_prefetch
                g = wns_next_prefetch
                if g >= NG2:
                    return
                wns_next_prefetch += 1
                wt = wnspool.tile([128, NCH // 2, 2, 2, O3], F8E4, tag="wns",
                                  name=f"wns_t{g}")
                nc.sync.dma_start(out=wt, in_=wns_d.ap()[g])
                wns_tiles[g] = wt

            def ns_units():
                # DoubleRow fp8e4: K=256 per matmul (chunk pair via the
                # [Ki, Ko=2, dim] interleave). One mini-group = 2 tokens,
                # one psS tile (token j in bank j = its own acc group).
                for g in range(NG2):
                    # keep the DMA prefetch ~4 groups ahead
                    while wns_next_prefetch < min(g + 4, NG2):
                        wns_prefetch()
                    wt = wns_tiles.pop(g)
                    ps = psS.tile([128, 2, QT], F32, tag="psS", name="ns_ps")
                    for j in range(2):
                        tok = 2 * g + j
                        for cp in range(NCH // 2):
                            nc.tensor.matmul(
                                ps[0:1, j, 0:O3],
                                lhsT=xnst_sb[:, 2 * cp:2 * cp + 2, tok:tok + 1],
                                rhs=wt[:, cp, j, :, :],
                                start=(cp == 0), stop=(cp == NCH // 2 - 1),
                                perf_mode=mybir.MatmulPerfMode.DoubleRow)
                    stg = stgpool.tile([1, 2, O3], BF16, tag="stg", name="stg")
                    nc.vector.tensor_copy(out=stg[0:1, :, :],
                                          in_=ps[0:1, 0:2, 0:O3])
                    nc.sync.dma_start(out=qkvns_sb[2 * g:2 * g + 2, :],
                                      in_=stg[0:1, :, :])
                    yield

            nsgen = ns_units()
            ns_left = NG2
            ns_finalized = [False]

            def ns_step(n=1):
                nonlocal ns_left
                for _ in range(n):
                    if ns_left > 0:
                        next(nsgen)
                        ns_left -= 1

            def ns_finalize():
                # Q_NS^T / K_NS^T via plain matmul against the 1/64-scaled
                # identity (transpose + descale in one); V_NS by 1/64 muls.
                for part, dest in ((0, qt_ns), (1, kt_ns)):
                    pst = psS.tile([128, 2, QT], F32, tag="psS", name="pst")
                    nc.tensor.matmul(
                        pst[:, 0, 0:64],
                        lhsT=qkvns_sb[0:64, part * 128:(part + 1) * 128],
                        rhs=ident_sb[:, :], start=True, stop=True)
                    nc.vector.tensor_copy(out=dest[:, :], in_=pst[:, 0, 0:64])
                for h in range(2):
                    nc.vector.tensor_scalar_mul(
                        v_ns[0:64, h, 0:64],
                        qkvns_sb[0:64, 256 + h * 64:256 + (h + 1) * 64],
                        1.0 / WNS_SCALE)

            # consume tiles 4,0,1,2,3 up-front (attention q-tile 0 deps):
            # tile4: K,Q,V0-3 = 6 units; tiles 0-3: K,V0-3 = 5 units each
            sproj_step(2)            # xt4 DMA + K4 + Q4 right behind ws
            wns_prefetch()
            wns_prefetch()
            sproj_step(4 + 4 * 5)
            # wout load after the critical-path constants
            nc.sync.dma_start(out=wout_sb, in_=wout_d.ap())

            # ---------------- main attention loop ----------------
            q_tiles = [(q0, min(QT, LQ - q0)) for q0 in range(0, LQ, QT)]
            pending_boundary = None

            for qt_i, (q0, qw) in enumerate(q_tiles):
                kc_count = min((QS + q0 + qw - 1) // 128 + 1, n_kc)
                is_last_qt = (q0 >= lqs)

                # everything q-tile qt_i reads (tiles <= 4+qt_i) must be
                # emitted before its chunk loop emits the readers
                need_done = min(26 + 6 * qt_i, 44)
                sproj_step(max(0, need_done - (44 - sproj_left)))

                # av tile allocated lazily (after the previous q-tile's
                # deferred boundary has emitted its reads of the old one).
                # Col-tiled: h0 -> psum partitions 0-63, h1 -> 64-127.
                # Softmax denominators accumulate on DVE in bf16 (even/odd
                # chunk split halves the rounding walk and deepens pipelining)
                av = [None]
                da = [None, None]
                da_used = [False, False]

                def alloc_av():
                    av[0] = psAV.tile([128, QT], F32, tag="av", name="av")
                    da[0] = dapool.tile([128, 2, QT], BF16, tag="da", name="da0")
                    da[1] = dapool.tile([128, 2, QT], BF16, tag="da", name="da1")
                    da_used[0] = da_used[1] = False

                def qk_pair(kc, ex_tiles):
                    # qlo: first unmasked query column for this key chunk
                    qlo = max(0, 128 * kc - (QS + q0)) if not is_last_qt else 0
                    is_ns_chunk = (kc == n_kc_s)
                    kw = LNS if is_ns_chunk else 128
                    ps = psS.tile([128, 2, QT], F32, tag="psS", name="ps_s")
                    for h in range(2):
                        hs = slice(h * 64, h * 64 + 64)
                        if is_ns_chunk:
                            k_src = kt_ns[hs, 0:kw]
                        else:
                            k_src = kt_s[hs, kc * 128:kc * 128 + kw]
                        if is_last_qt:
                            q_src = qt_ns[hs, q0 - lqs:q0 - lqs + qw]
                        else:
                            q_src = qt_s[hs, q0 + qlo:q0 + qw]
                        nc.tensor.matmul(ps[0:kw, h, qlo:qw], lhsT=k_src,
                                         rhs=q_src, start=True, stop=True)
                    ex = expool.tile([128, 2, QT], BF16, tag="exp", name="ex")
                    nc.scalar.activation(
                        out=ex[0:kw, :, qlo:qw], in_=ps[0:kw, :, qlo:qw],
                        func=mybir.ActivationFunctionType.Exp, scale=SCALE)
                    if 128 * kc + kw - 1 > QS + q0:
                        for h in range(2):
                            nc.gpsimd.affine_select(
                                out=ex[0:kw, h, qlo:qw], in_=ex[0:kw, h, qlo:qw],
                                compare_op=mybir.AluOpType.is_ge, fill=0.0,
                                base=QS + q0 + qlo - 128 * kc,
                                channel_multiplier=-1,
                                pattern=[[1, qw - qlo]])
                    ex_tiles[kc] = (ex, qlo)

                def dn_accum(kc, kw, ex, qlo, co=None):
                    # denominator accumulate; the even-parity chain runs on
                    # DVE, the odd-parity chain on GpSimd (both operands live
                    # in SBUF, and this splits ~70us of adds across the two
                    # engines). co is the column base in the (shared) ex tile
                    # for the NS q-tile grouped path.
                    src = ex[0:kw, :, qlo:qw] if co is None else \
                        ex[0:kw, :, co:co + qw]
                    a = kc % 2
                    eng = nc.vector if a == 0 else nc.gpsimd
                    dst = da[a][0:kw, :, qlo:qw]
                    if not da_used[a]:
                        # first chunk of each parity covers the full q range
                        # (qlo == 0) and all 128 key partitions
                        assert qlo == 0 and kw == 128
                        da_used[a] = True
                        eng.tensor_copy(out=dst, in_=src)
                    else:
                        eng.tensor_add(dst, da[a][0:kw, :, qlo:qw], src)

                def av_pair(kc, ex_tiles, last_kc):
                    is_ns_chunk = (kc == n_kc_s)
                    kw = LNS if is_ns_chunk else 128
                    ex, qlo = ex_tiles.pop(kc)
                    for h in range(2):
                        v_src = v_ns[0:kw, h, :] if is_ns_chunk \
                            else v_s[0:kw, kc, h, :]
                        nc.tensor.matmul(av[0][64 * h:64 * h + 64, qlo:qw],
                                         lhsT=v_src,
                                         rhs=ex[0:kw, h, qlo:qw],
                                         start=(kc == 0),
                                         stop=(kc == last_kc))
                    dn_accum(kc, kw, ex, qlo)

                def ns_qtile_group(c0, c1, ex_tiles):
                    # batch chunks [c0, c1) of the 64-wide NS q-tile into one
                    # scores tile at 64-col offsets; one exp per head group
                    ps = psS.tile([128, 2, QT], F32, tag="psS", name="ps_g")
                    ex = expool.tile([128, 2, QT], BF16, tag="exp", name="exg")
                    for kc in range(c0, c1):
                        is_ns_chunk = (kc == n_kc_s)
                        kw = LNS if is_ns_chunk else 128
                        co = 64 * (kc - c0)
                        for h in range(2):
                            hs = slice(h * 64, h * 64 + 64)
                            k_src = kt_ns[hs, 0:kw] if is_ns_chunk \
                                else kt_s[hs, kc * 128:kc * 128 + kw]
                            q_src = qt_ns[hs, 0:qw]
                            nc.tensor.matmul(ps[0:kw, h, co:co + qw],
                                             lhsT=k_src, rhs=q_src,
                                             start=True, stop=True)
                    wide = 64 * (c1 - c0)
                    kwmax = 128 if c1 - 1 < n_kc_s or c1 - c0 > 1 else LNS
                    nc.scalar.activation(
                        out=ex[0:kwmax, :, 0:wide], in_=ps[0:kwmax, :, 0:wide],
                        func=mybir.ActivationFunctionType.Exp, scale=SCALE)
                    for kc in range(c0, c1):
                        if kc == n_kc_s:
                            co = 64 * (kc - c0)
                            for h in range(2):
                                nc.gpsimd.affine_select(
                                    out=ex[0:LNS, h, co:co + qw],
                                    in_=ex[0:LNS, h, co:co + qw],
                                    compare_op=mybir.AluOpType.is_ge, fill=0.0,
                                    base=0, channel_multiplier=-1,
                                    pattern=[[1, qw]])
                    for kc in range(c0, c1):
                        ex_tiles[kc] = (ex, 64 * (kc - c0))

                def ns_av(kc, ex_tiles, last_kc):
                    is_ns_chunk = (kc == n_kc_s)
                    kw = LNS if is_ns_chunk else 128
                    ex, co = ex_tiles[kc]
                    for h in range(2):
                        v_src = v_ns[0:kw, h, :] if is_ns_chunk \
                            else v_s[0:kw, kc, h, :]
                        nc.tensor.matmul(av[0][64 * h:64 * h + 64, 0:qw],
                                         lhsT=v_src,
                                         rhs=ex[0:kw, h, co:co + qw],
                                         start=(kc == 0),
                                         stop=(kc == last_kc))
                    dn_accum(kc, kw, ex, 0, co)

                def make_boundary(av, da, q0, qw):
                    # normalize + W_out for a finished q-tile; deferred so it
                    # overlaps the next q-tile's first chunks instead of
                    # stalling the tensor queue
                    def boundary():
                        # reduce the bf16 denominator accumulators over the
                        # key partitions, broadcast 64-wide per head: one
                        # psum bank, h0 -> partitions 0-63, h1 -> 64-127
                        pbc = psS.tile([128, 2, QT], F32, tag="psS", name="pbc")
                        for h in range(2):
                            for a in range(2):
                                nc.tensor.matmul(
                                    pbc[64 * h:64 * h + 64, 0, 0:qw],
                                    lhsT=ones_bf[:, :],
                                    rhs=da[a][:, h, 0:qw],
                                    start=(a == 0), stop=(a == 1))
                        rcf = rcpool.tile([128, QT], F32, tag="rcf", name="rcf")
                        nc.vector.reciprocal_approx_fast(
                            out=rcf[:, 0:qw], in_=pbc[:, 0, 0:qw])
                        avtn = avtnpool.tile([128, QT], BF16, tag="avtn",
                                             name="avtn")
                        with nc.allow_low_precision(
                                reason="bf16 softmax normalize"):
                            nc.vector.tensor_mul(avtn[:, 0:qw], av[0][:, 0:qw],
                                                 rcf[:, 0:qw])
                        for qs in range(math.ceil(qw / 128)):
                            qsw = min(128, qw - qs * 128)
                            po = psS.tile([128, 2, QT], F32, tag="psS", name="po")
                            ot = outpool.tile([128, D], BF16, tag="out", name="ot")
                            for e in range(2):
                                nc.tensor.matmul(
                                    po[0:qsw, e, :],
                                    lhsT=avtn[:, qs * 128:qs * 128 + qsw],
                                    rhs=wout_sb[:, e * 512:(e + 1) * 512],
                                    start=True, stop=True)
                                # ScalarE does the psum->sbuf output cast;
                                # DVE is the busier engine here
                                nc.scalar.copy(
                                    out=ot[0:qsw, e * 512:(e + 1) * 512],
                                    in_=po[0:qsw, e, :])
                            nc.gpsimd.dma_start(
                                out=o_d.ap()[q0 + qs * 128:q0 + qs * 128 + qsw, :],
                                in_=ot[0:qsw, :])
                    return boundary

                ex_tiles = {}
                if is_last_qt:
                    # drain any remaining NS projection + finalize first
                    ns_step(ns_left)
                    sproj_step(sproj_left)
                    if not ns_finalized[0]:
                        ns_finalized[0] = True
                        ns_finalize()
                    if pending_boundary is not None:
                        pending_boundary()
                        pending_boundary = None
                    alloc_av()
                    groups = [(c0, min(c0 + 8, kc_count))
                              for c0 in range(0, kc_count, 8)]
                    for gi, (c0, c1) in enumerate(groups):
                        ns_qtile_group(c0, c1, ex_tiles)
                        if gi > 0:
                            p0, p1 = groups[gi - 1]
                            for kc in range(p0, p1):
                                ns_av(kc, ex_tiles, kc_count - 1)
                    p0, p1 = groups[-1]
                    for kc in range(p0, p1):
                        ns_av(kc, ex_tiles, kc_count - 1)
                    ex_tiles.clear()
                    make_boundary(av, da, q0, qw)()
                else:
                    for kc in range(kc_count):
                        qk_pair(kc, ex_tiles)
                        if kc == 1:
                            alloc_av()
                        if kc == 3 and pending_boundary is not None:
                            # deferred 3 chunks in: by now the previous
                            # q-tile's DVE normalize chain has drained, so
                            # these tensor-queue ops won't head-of-line block
                            pending_boundary()
                            pending_boundary = None
                        if kc > 0:
                            av_pair(kc - 1, ex_tiles, kc_count - 1)
                        # interleave fillers: NS mini-groups and the
                        # remaining s-proj tiles, at most one per chunk
                        if kc % 3 == 2:
                            ns_step()
                        elif kc % 3 == 0 and kc > 0:
                            sproj_step()
                        if ns_left == 0 and sproj_left == 0 \
                                and not ns_finalized[0]:
                            ns_finalized[0] = True
                            ns_finalize()
                    av_pair(kc_count - 1, ex_tiles, kc_count - 1)
                    pending_boundary = make_boundary(av, da, q0, qw)

    nc.compile()
    return nc


_NC_CACHE = {}


def _get_program():
    if "nc" not in _NC_CACHE:
        _NC_CACHE["nc"] = build_program()
    return _NC_CACHE["nc"]


def _prep_core(c, xt, xnst, W_S, W_NS, W_out):
    """Host-side shard prep for core c (heads 2c, 2c+1)."""
    h0 = 2 * c * DH
    cols = np.r_[h0:h0 + HPC * DH,
                 D + h0:D + h0 + HPC * DH,
                 2 * D + h0:2 * D + h0 + HPC * DH]
    ws = W_S[:, cols].reshape(NCH, 128, O3).transpose(1, 0, 2)
    ws = np.ascontiguousarray(ws.astype(ml_dtypes.bfloat16))
    # wns: [LNS, D, O3cols] -> [NG2, 128, NCH//2, 2tok, 2ko, O3] fp8 (x64)
    wns = W_NS[:, :, cols] * WNS_SCALE
    # [tok, d, o] with d = 256*cp + 128*ko + p -> [g, p, cp, tok2, ko, o]
    wns = wns.reshape(NG2, 2, NCH // 2, 2, 128, O3).transpose(0, 4, 2, 1, 3, 5)
    wns = np.ascontiguousarray(wns.astype(ml_dtypes.float8_e4m3))
    wout = np.ascontiguousarray(
        W_out[h0:h0 + HPC * DH].astype(ml_dtypes.bfloat16))
    return {"xt": xt, "xnst": xnst, "ws": ws, "wns": wns, "wout": wout,
            "vones": np.ones((128, 64), dtype=ml_dtypes.bfloat16)}


def prep_in_maps(np_inputs):
    """Host-side shard prep for all cores (used by kernel() and test timing)."""
    x = np.asarray(np_inputs["x"], dtype=np.float32)
    W_S = np.asarray(np_inputs["W_S"], dtype=np.float32)
    W_NS = np.asarray(np_inputs["W_NS"], dtype=np.float32)
    W_out = np.asarray(np_inputs["W_out"], dtype=np.float32)
    xs = x[0]
    xt = xs[:LS].T.reshape(NCH, 128, NT, ST).transpose(2, 1, 0, 3)
    xt = np.ascontiguousarray(xt.astype(ml_dtypes.bfloat16))
    xnst = xs[LS:].T.reshape(NCH, 128, LNS).transpose(1, 0, 2)
    xnst = np.ascontiguousarray(xnst.astype(ml_dtypes.float8_e4m3))
    with ThreadPoolExecutor(max_workers=N_CORES) as ex:
        return list(ex.map(
            lambda c: _prep_core(c, xt, xnst, W_S, W_NS, W_out),
            range(N_CORES)))


def kernel(x, W_S, W_NS, W_out, L_S=None, query_start=None, **_unused):
    x = np.asarray(x, dtype=np.float32)
    W_S = np.asarray(W_S, dtype=np.float32)
    W_NS = np.asarray(W_NS, dtype=np.float32)
    W_out = np.asarray(W_out, dtype=np.float32)
    if L_S is not None:
        assert int(L_S) == LS, f"kernel hardcodes L_S={LS}, got {int(L_S)}"
    if query_start is not None:
        assert int(query_start) == QS, \
            f"kernel hardcodes query_start={QS}, got {int(query_start)}"
    assert x.shape == (1, LS + LNS, D)

    nc = _get_program()
    in_maps = prep_in_maps({"x": x, "W_S": W_S, "W_NS": W_NS, "W_out": W_out})

    res = None
    for attempt in range(3):
        try:
            res = run_bass_kernel_spmd(nc, in_maps, list(range(N_CORES)))
            break
        except Exception:
            if attempt == 2:
                raise
            import time
            time.sleep(100)
    out = np.zeros((LQ, D), dtype=np.float32)
    for r in res.results:
        out += np.asarray(r["o"], dtype=np.float32)
    return out.reshape(1, LQ, D)


if __name__ == "__main__":
    rng = np.random.default_rng(0)
    ins = {
        "x": rng.standard_normal((1, LS + LNS, D), dtype=np.float32),
        "W_S": rng.standard_normal((D, 3 * D), dtype=np.float32) * 0.02,
        "W_NS": rng.standard_normal((LNS, D, 3 * D), dtype=np.float32) * 0.02,
        "W_out": rng.standard_normal((D, D), dtype=np.float32) * 0.03,
        "L_S": LS, "query_start": QS,
    }
    out = kernel(**ins)
    print("kernel out shape:", out.shape, "finite:", np.isfinite(out).all())


# revision 40
# speedup vs baseline: 1.0426x; 1.0426x over previous
"""Trainium2 Bass kernel for MixedCausalAttention (16 heads, d=1024, L_S=4096, L_NS=64).

Sharding: tensor-parallel over heads - 2 heads per core x 8 cores.
Each core computes qkv projections for its 2 heads, causal attention, and a
partial W_out product over its 128 output feature rows (head-stacked K=128
matmul). The host sums the 8 partial (2112, 1024) bf16 outputs in f32.

v3 redesign vs v2:
- NS projection fully interleaved into the attention chunk loop as 2-token
  mini-groups (one psS tile = 2 banks, one accumulation group per bank),
  eliminating the serial NS phases that left ACT idle ~100us.
- W_NS descale (x64) folded into the PE transpose identity (x 1/64) and the
  V_NS staging muls - no more per-token single-partition tensor_scalar work.
- qkv_NS staged bf16 via one strided CAST + sbuf->sbuf DMA per mini-group.
- Reciprocal merged to one [2, QT] instr per q-tile (was 2x single-partition).
- V staging: single v_s tile, one CAST per 128-key sub (both heads at once).
- PSUM: psS bufs=3 (6 banks, shared by scores/proj/NS/po/pbc) + av 2 banks.
- DMA order tuned: ws + early x tiles first, wout last, wns streamed with
  bounded prefetch so compute can start ~3us in.
"""

import os
import sys
import math
from concurrent.futures import ThreadPoolExecutor

for _p in ("/opt/trn_rl_repo", "/root/.axon_site/_ro/trn_rl_repo"):
    if os.path.isdir(_p) and _p not in sys.path:
        sys.path.insert(0, _p)

import numpy as np
import ml_dtypes

import concourse.bass as bass
import concourse.mybir as mybir
import concourse.tile as tile
from concourse import bacc
from concourse.bass_utils import run_bass_kernel_spmd

F32 = mybir.dt.float32
BF16 = mybir.dt.bfloat16
F8E4 = mybir.dt.float8e4

N_CORES = 8
D = 1024
H = 16
DH = 64
HPC = H // N_CORES          # heads per core = 2
O3 = 3 * DH * HPC           # 384 qkv output cols per core
LNS = 64
LS = 4096
QS = 2048                   # query_start
LQ = LS - QS + LNS          # 2112 queries
NCH = D // 128              # 8 contraction chunks
ST = 512                    # s-tile width for projections
NT = LS // ST               # 8 s-tiles
QT = 512                    # q-tile width for attention
NG2 = LNS // 2              # 32 NS token mini-groups (2 tokens each)
SCALE = DH ** -0.5
WNS_SCALE = 64.0

n_kc_s = LS // 128          # 32 S key chunks
n_kc = n_kc_s + 1           # + NS chunk
lqs = LS - QS               # 2048 S-query columns


def build_program(repeat=1):
    nc = bacc.Bacc("TRN2", target_bir_lowering=False, debug=False,
                   num_devices=N_CORES)

    xt_d = nc.dram_tensor("xt", [NT, 128, NCH, ST], BF16, kind="ExternalInput")
    xnst_d = nc.dram_tensor("xnst", [128, NCH, LNS], F8E4, kind="ExternalInput")
    ws_d = nc.dram_tensor("ws", [128, NCH, O3], BF16, kind="ExternalInput")
    wns_d = nc.dram_tensor("wns", [NG2, 128, NCH // 2, 2, 2, O3], F8E4,
                           kind="ExternalInput")
    wout_d = nc.dram_tensor("wout", [128, D], BF16, kind="ExternalInput")
    vones_d = nc.dram_tensor("vones", [128, 64], BF16, kind="ExternalInput")
    o_d = nc.dram_tensor("o", [LQ, D], BF16, kind="ExternalOutput")

    with tile.TileContext(nc) as tc:
      for _rep in range(repeat):
        import contextlib
        ctx = contextlib.ExitStack()
        with ctx:
            const = ctx.enter_context(tc.tile_pool(name="const", bufs=1))
            store = ctx.enter_context(tc.tile_pool(name="store", bufs=1))

            # --- constants ---
            # ws first: it gates the very first projection matmul. All the
            # all-ones tiles are memset on gpsimd instead of DMA'd (the
            # strided ones-scatter DMAs took ~3us each to issue on sync).
            ws_sb = const.tile([128, NCH, O3], BF16)
            nc.sync.dma_start(out=ws_sb, in_=ws_d.ap())
            ones_bf = const.tile([128, 64], BF16)
            nc.gpsimd.memset(ones_bf[:, :], 1.0)
            xnst_sb = const.tile([128, NCH, LNS], F8E4)
            nc.sync.dma_start(out=xnst_sb, in_=xnst_d.ap())
            # scaled identity (1/WNS_SCALE on the diagonal) for the NS
            # transposes - folds the W_NS descale into the PE transpose
            ident_sb = const.tile([64, 64], BF16)
            nc.gpsimd.memset(ident_sb[:, :], 0.0)
            nc.gpsimd.affine_select(
                out=ident_sb[:, :], in_=ident_sb[:, :],
                compare_op=mybir.AluOpType.not_equal, fill=1.0 / WNS_SCALE,
                base=0, channel_multiplier=1, pattern=[[-1, 64]])
            wout_sb = const.tile([128, D], BF16)

            # --- persistent activation storage (all bf16) ---
            kt_s = store.tile([128, LS], BF16)      # K^T (h0 rows 0-63, h1 64-127)
            qt_s = store.tile([128, lqs], BF16)     # Q^T, S part
            kt_ns = store.tile([128, LNS], BF16)
            qt_ns = store.tile([128, LNS], BF16)
            # V natural layout: [keys, kc, head, dh]
            v_s = store.tile([128, n_kc_s, HPC, DH], BF16)
            v_ns = store.tile([64, HPC, DH], BF16)
            qkvns_sb = store.tile([LNS, O3], BF16)  # NS qkv rows (x64 scaled)

            xpool = ctx.enter_context(tc.tile_pool(name="xpool", bufs=3))
            wnspool = ctx.enter_context(tc.tile_pool(name="wnspool", bufs=6))
            expool = ctx.enter_context(tc.tile_pool(name="expool", bufs=4))
            dapool = ctx.enter_context(tc.tile_pool(name="dapool", bufs=4))
            rcpool = ctx.enter_context(tc.tile_pool(name="rcpool", bufs=2))
            avtnpool = ctx.enter_context(tc.tile_pool(name="avtnpool", bufs=2))
            outpool = ctx.enter_context(tc.tile_pool(name="outpool", bufs=2))
            stgpool = ctx.enter_context(tc.tile_pool(name="stgpool", bufs=2))
            psS = ctx.enter_context(tc.tile_pool(name="psS", bufs=3, space="PSUM"))
            psAV = ctx.enter_context(tc.tile_pool(name="psAV", bufs=2, space="PSUM"))

            # ---------------- S-token projection units ----------------
            # Emitted lazily; tiles in dependency order [4,0,1,2,3] then
            # [5,6,7] interleaved into the attention chunk loops.
            def sproj_units():
                first = True
                for t in (4, 0, 1, 2, 3, 5, 6, 7):
                    s0 = t * ST
                    xt_t = xpool.tile([128, NCH, ST], BF16, tag="xt", name="xt_t")
                    if first:
                        # split the first tile's load so the first projection
                        # matmul (which reads ci=0 first) starts sooner
                        first = False
                        nc.sync.dma_start(out=xt_t[:, 0:2, :],
                                          in_=xt_d.ap()[t][:, 0:2, :])
                        nc.sync.dma_start(out=xt_t[:, 2:NCH, :],
                                          in_=xt_d.ap()[t][:, 2:NCH, :])
                    else:
                        nc.sync.dma_start(out=xt_t, in_=xt_d.ap()[t])
                    # K^T (and Q^T for t>=4): W_S columns stationary, x moving
                    jobs = [(1, kt_s, s0)]
                    if t >= 4:
                        jobs.append((0, qt_s, s0 - QS))
                    for mi, dest, dcol in jobs:
                        ps = psS.tile([128, 2, QT], F32, tag="psS", name="ps_kq")
                        for ci in range(NCH):
                            nc.tensor.matmul(
                                ps[:, 0, :],
                                lhsT=ws_sb[:, ci, mi * 128:(mi + 1) * 128],
                                rhs=xt_t[:, ci, :],
                                start=(ci == 0), stop=(ci == NCH - 1))
                        nc.vector.tensor_copy(out=dest[:, dcol:dcol + ST],
                                              in_=ps[:, 0, :])
                        yield
                    # V natural: x^T chunk stationary, W_S V-cols moving
                    for sub in range(ST // 128):
                        kc = 4 * t + sub
                        psv = psS.tile([128, 2, QT], F32, tag="psS", name="psv")
                        for ci in range(NCH):
                            nc.tensor.matmul(
                                psv[:, 0, 0:128],
                                lhsT=xt_t[:, ci, sub * 128:(sub + 1) * 128],
                                rhs=ws_sb[:, ci, 256:384],
                                start=(ci == 0), stop=(ci == NCH - 1))
                        # both heads in one strided CAST
                        nc.vector.tensor_copy(
                            out=v_s[:, kc, :, 0:64],
                            in_=psv[:, 0, 0:128])
                        yield

            sproj = sproj_units()
            sproj_left = 8 * 4 + 8 + 4   # 44 units total

            def sproj_step(n=1):
                nonlocal sproj_left
                for _ in range(n):
                    if sproj_left > 0:
                        next(sproj)
                        sproj_left -= 1

            # ---------------- NS-token projection mini-groups ----------------
            wns_tiles = {}
            wns_next_prefetch = 0

            def wns_prefetch():
                nonlocal wns_next_prefetch
                g = wns_next_prefetch
                if g >= NG2:
                    return
                wns_next_prefetch += 1
                wt = wnspool.tile([128, NCH // 2, 2, 2, O3], F8E4, tag="wns",
                                  name=f"wns_t{g}")
                nc.sync.dma_start(out=wt, in_=wns_d.ap()[g])
                wns_tiles[g] = wt

            def ns_units():
                # DoubleRow fp8e4: K=256 per matmul (chunk pair via the
                # [Ki, Ko=2, dim] interleave). One mini-group = 2 tokens,
                # one psS tile (token j in bank j = its own acc group).
                for g in range(NG2):
                    # keep the DMA prefetch ~4 groups ahead
                    while wns_next_prefetch < min(g + 4, NG2):
                        wns_prefetch()
                    wt = wns_tiles.pop(g)
                    ps = psS.tile([128, 2, QT], F32, tag="psS", name="ns_ps")
                    for j in range(2):
                        tok = 2 * g + j
                        for cp in range(NCH // 2):
                            nc.tensor.matmul(
                                ps[0:1, j, 0:O3],
                                lhsT=xnst_sb[:, 2 * cp:2 * cp + 2, tok:tok + 1],
                                rhs=wt[:, cp, j, :, :],
                                start=(cp == 0), stop=(cp == NCH // 2 - 1),
                                perf_mode=mybir.MatmulPerfMode.DoubleRow)
                    stg = stgpool.tile([1, 2, O3], BF16, tag="stg", name="stg")
                    nc.vector.tensor_copy(out=stg[0:1, :, :],
                                          in_=ps[0:1, 0:2, 0:O3])
                    nc.sync.dma_start(out=qkvns_sb[2 * g:2 * g + 2, :],
                                      in_=stg[0:1, :, :])
                    yield

            nsgen = ns_units()
            ns_left = NG2
            ns_finalized = [False]

            def ns_step(n=1):
                nonlocal ns_left
                for _ in range(n):
                    if ns_left > 0:
                        next(nsgen)
                        ns_left -= 1

            def ns_finalize():
                # Q_NS^T / K_NS^T via plain matmul against the 1/64-scaled
                # identity (transpose + descale in one); V_NS by 1/64 muls.
                for part, dest in ((0, qt_ns), (1, kt_ns)):
                    pst = psS.tile([128, 2, QT], F32, tag="psS", name="pst")
                    nc.tensor.matmul(
                        pst[:, 0, 0:64],
                        lhsT=qkvns_sb[0:64, part * 128:(part + 1) * 128],
                        rhs=ident_sb[:, :], start=True, stop=True)
                    nc.vector.tensor_copy(out=dest[:, :], in_=pst[:, 0, 0:64])
                for h in range(2):
                    nc.vector.tensor_scalar_mul(
                        v_ns[0:64, h, 0:64],
                        qkvns_sb[0:64, 256 + h * 64:256 + (h + 1) * 64],
                        1.0 / WNS_SCALE)

            # consume tiles 4,0,1,2,3 up-front (attention q-tile 0 deps):
            # tile4: K,Q,V0-3 = 6 units; tiles 0-3: K,V0-3 = 5 units each
            sproj_step(2)            # xt4 DMA + K4 + Q4 right behind ws
            wns_prefetch()
            wns_prefetch()
            sproj_step(4 + 4 * 5)
            # wout load after the critical-path constants
            nc.sync.dma_start(out=wout_sb, in_=wout_d.ap())

            # ---------------- main attention loop ----------------
            q_tiles = [(q0, min(QT, LQ - q0)) for q0 in range(0, LQ, QT)]
            pending_boundary = None

            for qt_i, (q0, qw) in enumerate(q_tiles):
                kc_count = min((QS + q0 + qw - 1) // 128 + 1, n_kc)
                is_last_qt = (q0 >= lqs)

                # everything q-tile qt_i reads (tiles <= 4+qt_i) must be
                # emitted before its chunk loop emits the readers
                need_done = min(26 + 6 * qt_i, 44)
                sproj_step(max(0, need_done - (44 - sproj_left)))

                # av tile allocated lazily (after the previous q-tile's
                # deferred boundary has emitted its reads of the old one).
                # Col-tiled: h0 -> psum partitions 0-63, h1 -> 64-127.
                # Softmax denominators accumulate on DVE in bf16 (even/odd
                # chunk split halves the rounding walk and deepens pipelining)
                av = [None]
                da = [None, None]
                da_used = [False, False]

                def alloc_av():
                    av[0] = psAV.tile([128, QT], F32, tag="av", name="av")
                    da[0] = dapool.tile([128, 2, QT], BF16, tag="da", name="da0")
                    da[1] = dapool.tile([128, 2, QT], BF16, tag="da", name="da1")
                    da_used[0] = da_used[1] = False

                def qk_pair(kc, ex_tiles):
                    # qlo: first unmasked query column for this key chunk
                    qlo = max(0, 128 * kc - (QS + q0)) if not is_last_qt else 0
                    is_ns_chunk = (kc == n_kc_s)
                    kw = LNS if is_ns_chunk else 128
                    ps = psS.tile([128, 2, QT], F32, tag="psS", name="ps_s")
                    for h in range(2):
                        hs = slice(h * 64, h * 64 + 64)
                        if is_ns_chunk:
                            k_src = kt_ns[hs, 0:kw]
                        else:
                            k_src = kt_s[hs, kc * 128:kc * 128 + kw]
                        if is_last_qt:
                            q_src = qt_ns[hs, q0 - lqs:q0 - lqs + qw]
                        else:
                            q_src = qt_s[hs, q0 + qlo:q0 + qw]
                        nc.tensor.matmul(ps[0:kw, h, qlo:qw], lhsT=k_src,
                                         rhs=q_src, start=True, stop=True)
                    ex = expool.tile([128, 2, QT], BF16, tag="exp", name="ex")
                    nc.scalar.activation(
                        out=ex[0:kw, :, qlo:qw], in_=ps[0:kw, :, qlo:qw],
                        func=mybir.ActivationFunctionType.Exp, scale=SCALE)
                    if 128 * kc + kw - 1 > QS + q0:
                        for h in range(2):
                            nc.gpsimd.affine_select(
                                out=ex[0:kw, h, qlo:qw], in_=ex[0:kw, h, qlo:qw],
                                compare_op=mybir.AluOpType.is_ge, fill=0.0,
                                base=QS + q0 + qlo - 128 * kc,
                                channel_multiplier=-1,
                                pattern=[[1, qw - qlo]])
                    ex_tiles[kc] = (ex, qlo)

                def dn_accum(kc, kw, ex, qlo, co=None):
                    # denominator accumulate; the even-parity chain runs on
                    # DVE, the odd-parity chain on GpSimd (both operands live
                    # in SBUF, and this splits ~70us of adds across the two
                    # engines). co is the column base in the (shared) ex tile
                    # for the NS q-tile grouped path.
                    src = ex[0:kw, :, qlo:qw] if co is None else \
                        ex[0:kw, :, co:co + qw]
                    a = kc % 2
                    eng = nc.vector
                    dst = da[a][0:kw, :, qlo:qw]
                    if not da_used[a]:
                        # first chunk of each parity covers the full q range
                        # (qlo == 0) and all 128 key partitions
                        assert qlo == 0 and kw == 128
                        da_used[a] = True
                        eng.tensor_copy(out=dst, in_=src)
                    else:
                        eng.tensor_add(dst, da[a][0:kw, :, qlo:qw], src)

                def av_pair(kc, ex_tiles, last_kc):
                    is_ns_chunk = (kc == n_kc_s)
                    kw = LNS if is_ns_chunk else 128
                    ex, qlo = ex_tiles.pop(kc)
                    for h in range(2):
                        v_src = v_ns[0:kw, h, :] if is_ns_chunk \
                            else v_s[0:kw, kc, h, :]
                        nc.tensor.matmul(av[0][64 * h:64 * h + 64, qlo:qw],
                                         lhsT=v_src,
                                         rhs=ex[0:kw, h, qlo:qw],
                                         start=(kc == 0),
                                         stop=(kc == last_kc))
                    dn_accum(kc, kw, ex, qlo)

                def ns_qtile_group(c0, c1, ex_tiles):
                    # batch chunks [c0, c1) of the 64-wide NS q-tile into one
                    # scores tile at 64-col offsets; one exp per head group
                    ps = psS.tile([128, 2, QT], F32, tag="psS", name="ps_g")
                    ex = expool.tile([128, 2, QT], BF16, tag="exp", name="exg")
                    for kc in range(c0, c1):
                        is_ns_chunk = (kc == n_kc_s)
                        kw = LNS if is_ns_chunk else 128
                        co = 64 * (kc - c0)
                        for h in range(2):
                            hs = slice(h * 64, h * 64 + 64)
                            k_src = kt_ns[hs, 0:kw] if is_ns_chunk \
                                else kt_s[hs, kc * 128:kc * 128 + kw]
                            q_src = qt_ns[hs, 0:qw]
                            nc.tensor.matmul(ps[0:kw, h, co:co + qw],
                                             lhsT=k_src, rhs=q_src,
                                             start=True, stop=True)
                    wide = 64 * (c1 - c0)
                    kwmax = 128 if c1 - 1 < n_kc_s or c1 - c0 > 1 else LNS
                    nc.scalar.activation(
                        out=ex[0:kwmax, :, 0:wide], in_=ps[0:kwmax, :, 0:wide],
                        func=mybir.ActivationFunctionType.Exp, scale=SCALE)
                    for kc in range(c0, c1):
                        if kc == n_kc_s:
                            co = 64 * (kc - c0)
                            for h in range(2):
                                nc.gpsimd.affine_select(
                                    out=ex[0:LNS, h, co:co + qw],
                                    in_=ex[0:LNS, h, co:co + qw],
                                    compare_op=mybir.AluOpType.is_ge, fill=0.0,
                                    base=0, channel_multiplier=-1,
                                    pattern=[[1, qw]])
                    for kc in range(c0, c1):
                        ex_tiles[kc] = (ex, 64 * (kc - c0))

                def ns_av(kc, ex_tiles, last_kc):
                    is_ns_chunk = (kc == n_kc_s)
                    kw = LNS if is_ns_chunk else 128
                    ex, co = ex_tiles[kc]
                    for h in range(2):
                        v_src = v_ns[0:kw, h, :] if is_ns_chunk \
                            else v_s[0:kw, kc, h, :]
                        nc.tensor.matmul(av[0][64 * h:64 * h + 64, 0:qw],
                                         lhsT=v_src,
                                         rhs=ex[0:kw, h, co:co + qw],
                                         start=(kc == 0),
                                         stop=(kc == last_kc))
                    dn_accum(kc, kw, ex, 0, co)

                def make_boundary(av, da, q0, qw):
                    # normalize + W_out for a finished q-tile; deferred so it
                    # overlaps the next q-tile's first chunks instead of
                    # stalling the tensor queue
                    def boundary():
                        # reduce the bf16 denominator accumulators over the
                        # key partitions, broadcast 64-wide per head: one
                        # psum bank, h0 -> partitions 0-63, h1 -> 64-127
                        pbc = psS.tile([128, 2, QT], F32, tag="psS", name="pbc")
                        for h in range(2):
                            for a in range(2):
                                nc.tensor.matmul(
                                    pbc[64 * h:64 * h + 64, 0, 0:qw],
                                    lhsT=ones_bf[:, :],
                                    rhs=da[a][:, h, 0:qw],
                                    start=(a == 0), stop=(a == 1))
                        rcf = rcpool.tile([128, QT], F32, tag="rcf", name="rcf")
                        nc.vector.reciprocal_approx_fast(
                            out=rcf[:, 0:qw], in_=pbc[:, 0, 0:qw])
                        avtn = avtnpool.tile([128, QT], BF16, tag="avtn",
                                             name="avtn")
                        with nc.allow_low_precision(
                                reason="bf16 softmax normalize"):
                            nc.vector.tensor_mul(avtn[:, 0:qw], av[0][:, 0:qw],
                                                 rcf[:, 0:qw])
                        for qs in range(math.ceil(qw / 128)):
                            qsw = min(128, qw - qs * 128)
                            po = psS.tile([128, 2, QT], F32, tag="psS", name="po")
                            ot = outpool.tile([128, D], BF16, tag="out", name="ot")
                            for e in range(2):
                                nc.tensor.matmul(
                                    po[0:qsw, e, :],
                                    lhsT=avtn[:, qs * 128:qs * 128 + qsw],
                                    rhs=wout_sb[:, e * 512:(e + 1) * 512],
                                    start=True, stop=True)
                                # ScalarE does the psum->sbuf output cast;
                                # DVE is the busier engine here
                                nc.scalar.copy(
                                    out=ot[0:qsw, e * 512:(e + 1) * 512],
                                    in_=po[0:qsw, e, :])
                            nc.gpsimd.dma_start(
                                out=o_d.ap()[q0 + qs * 128:q0 + qs * 128 + qsw, :],
                                in_=ot[0:qsw, :])
                    return boundary

                ex_tiles = {}
                if is_last_qt:
                    # drain any remaining NS projection + finalize first
                    ns_step(ns_left)
                    sproj_step(sproj_left)
                    if not ns_finalized[0]:
                        ns_finalized[0] = True
                        ns_finalize()
                    if pending_boundary is not None:
                        pending_boundary()
                        pending_boundary = None
                    alloc_av()
                    groups = [(c0, min(c0 + 8, kc_count))
                              for c0 in range(0, kc_count, 8)]
                    for gi, (c0, c1) in enumerate(groups):
                        ns_qtile_group(c0, c1, ex_tiles)
                        if gi > 0:
                            p0, p1 = groups[gi - 1]
                            for kc in range(p0, p1):
                                ns_av(kc, ex_tiles, kc_count - 1)
                    p0, p1 = groups[-1]
                    for kc in range(p0, p1):
                        ns_av(kc, ex_tiles, kc_count - 1)
                    ex_tiles.clear()
                    make_boundary(av, da, q0, qw)()
                else:
                    for kc in range(kc_count):
                        qk_pair(kc, ex_tiles)
                        if kc == 1:
                            alloc_av()
                        if kc == 3 and pending_boundary is not None:
                            # deferred 3 chunks in: by now the previous
                            # q-tile's DVE normalize chain has drained, so
                            # these tensor-queue ops won't head-of-line block
                            pending_boundary()
                            pending_boundary = None
                        if kc > 0:
                            av_pair(kc - 1, ex_tiles, kc_count - 1)
                        # interleave fillers: NS mini-groups and the
                        # remaining s-proj tiles, at most one per chunk
                        if kc % 3 == 2:
                            ns_step()
                        elif kc % 3 == 0 and kc > 0:
                            sproj_step()
                        if ns_left == 0 and sproj_left == 0 \
                                and not ns_finalized[0]:
                            ns_finalized[0] = True
                            ns_finalize()
                    av_pair(kc_count - 1, ex_tiles, kc_count - 1)
                    pending_boundary = make_boundary(av, da, q0, qw)

    nc.compile()
    return nc


_NC_CACHE = {}


def _get_program():
    if "nc" not in _NC_CACHE:
        _NC_CACHE["nc"] = build_program()
    return _NC_CACHE["nc"]


def _prep_core(c, xt, xnst, W_S, W_NS, W_out):
    """Host-side shard prep for core c (heads 2c, 2c+1)."""
    h0 = 2 * c * DH
    cols = np.r_[h0:h0 + HPC * DH,
                 D + h0:D + h0 + HPC * DH,
                 2 * D + h0:2 * D + h0 + HPC * DH]
    ws = W_S[:, cols].reshape(NCH, 128, O3).transpose(1, 0, 2)
    ws = np.ascontiguousarray(ws.astype(ml_dtypes.bfloat16))
    # wns: [LNS, D, O3cols] -> [NG2, 128, NCH//2, 2tok, 2ko, O3] fp8 (x64)
    wns = W_NS[:, :, cols] * WNS_SCALE
    # [tok, d, o] with d = 256*cp + 128*ko + p -> [g, p, cp, tok2, ko, o]
    wns = wns.reshape(NG2, 2, NCH // 2, 2, 128, O3).transpose(0, 4, 2, 1, 3, 5)
    wns = np.ascontiguousarray(wns.astype(ml_dtypes.float8_e4m3))
    wout = np.ascontiguousarray(
        W_out[h0:h0 + HPC * DH].astype(ml_dtypes.bfloat16))
    return {"xt": xt, "xnst": xnst, "ws": ws, "wns": wns, "wout": wout,
            "vones": np.ones((128, 64), dtype=ml_dtypes.bfloat16)}


def prep_in_maps(np_inputs):
    """Host-side shard prep for all cores (used by kernel() and test timing)."""
    x = np.asarray(np_inputs["x"], dtype=np.float32)
    W_S = np.asarray(np_inputs["W_S"], dtype=np.float32)
    W_NS = np.asarray(np_inputs["W_NS"], dtype=np.float32)
    W_out = np.asarray(np_inputs["W_out"], dtype=np.float32)
    xs = x[0]
    xt = xs[:LS].T.reshape(NCH, 128, NT, ST).transpose(2, 1, 0, 3)
    xt = np.ascontiguousarray(xt.astype(ml_dtypes.bfloat16))
    xnst = xs[LS:].T.reshape(NCH, 128, LNS).transpose(1, 0, 2)
    xnst = np.ascontiguousarray(xnst.astype(ml_dtypes.float8_e4m3))
    with ThreadPoolExecutor(max_workers=N_CORES) as ex:
        return list(ex.map(
            lambda c: _prep_core(c, xt, xnst, W_S, W_NS, W_out),
            range(N_CORES)))


def kernel(x, W_S, W_NS, W_out, L_S=None, query_start=None, **_unused):
    x = np.asarray(x, dtype=np.float32)
    W_S = np.asarray(W_S, dtype=np.float32)
    W_NS = np.asarray(W_NS, dtype=np.float32)
    W_out = np.asarray(W_out, dtype=np.float32)
    if L_S is not None:
        assert int(L_S) == LS, f"kernel hardcodes L_S={LS}, got {int(L_S)}"
    if query_start is not None:
        assert int(query_start) == QS, \
            f"kernel hardcodes query_start={QS}, got {int(query_start)}"
    assert x.shape == (1, LS + LNS, D)

    nc = _get_program()
    in_maps = prep_in_maps({"x": x, "W_S": W_S, "W_NS": W_NS, "W_out": W_out})

    res = None
    for attempt in range(3):
        try:
            res = run_bass_kernel_spmd(nc, in_maps, list(range(N_CORES)))
            break
        except Exception:
            if attempt == 2:
                raise
            import time
            time.sleep(100)
    out = np.zeros((LQ, D), dtype=np.float32)
    for r in res.results:
        out += np.asarray(r["o"], dtype=np.float32)
    return out.reshape(1, LQ, D)


if __name__ == "__main__":
    rng = np.random.default_rng(0)
    ins = {
        "x": rng.standard_normal((1, LS + LNS, D), dtype=np.float32),
        "W_S": rng.standard_normal((D, 3 * D), dtype=np.float32) * 0.02,
        "W_NS": rng.standard_normal((LNS, D, 3 * D), dtype=np.float32) * 0.02,
        "W_out": rng.standard_normal((D, D), dtype=np.float32) * 0.03,
        "L_S": LS, "query_start": QS,
    }
    out = kernel(**ins)
    print("kernel out shape:", out.shape, "finite:", np.isfinite(out).all())


# revision 47
# speedup vs baseline: 1.1047x; 1.0595x over previous
"""Trainium2 Bass kernel for MixedCausalAttention (16 heads, d=1024, L_S=4096, L_NS=64).

Sharding: tensor-parallel over heads - 2 heads per core x 8 cores.
Each core computes qkv projections for its 2 heads, causal attention, and a
partial W_out product over its 128 output feature rows (head-stacked K=128
matmul). The host sums the 8 partial (2112, 1024) bf16 outputs in f32.

v3 redesign vs v2:
- NS projection fully interleaved into the attention chunk loop as 2-token
  mini-groups (one psS tile = 2 banks, one accumulation group per bank),
  eliminating the serial NS phases that left ACT idle ~100us.
- W_NS descale (x64) folded into the PE transpose identity (x 1/64) and the
  V_NS staging muls - no more per-token single-partition tensor_scalar work.
- qkv_NS staged bf16 via one strided CAST + sbuf->sbuf DMA per mini-group.
- Reciprocal merged to one [2, QT] instr per q-tile (was 2x single-partition).
- V staging: single v_s tile, one CAST per 128-key sub (both heads at once).
- PSUM: psS bufs=3 (6 banks, shared by scores/proj/NS/po/pbc) + av 2 banks.
- DMA order tuned: ws + early x tiles first, wout last, wns streamed with
  bounded prefetch so compute can start ~3us in.
"""

import os
import sys
import math
from concurrent.futures import ThreadPoolExecutor

for _p in ("/opt/trn_rl_repo", "/root/.axon_site/_ro/trn_rl_repo"):
    if os.path.isdir(_p) and _p not in sys.path:
        sys.path.insert(0, _p)

import numpy as np
import ml_dtypes

import concourse.bass as bass
import concourse.mybir as mybir
import concourse.tile as tile
from concourse import bacc
from concourse.bass_utils import run_bass_kernel_spmd

F32 = mybir.dt.float32
BF16 = mybir.dt.bfloat16
F8E4 = mybir.dt.float8e4

N_CORES = 8
D = 1024
H = 16
DH = 64
HPC = H // N_CORES          # heads per core = 2
O3 = 3 * DH * HPC           # 384 qkv output cols per core
LNS = 64
LS = 4096
QS = 2048                   # query_start
LQ = LS - QS + LNS          # 2112 queries
NCH = D // 128              # 8 contraction chunks
ST = 512                    # s-tile width for projections
NT = LS // ST               # 8 s-tiles
QT = 512                    # q-tile width for attention
NG2 = LNS // 2              # 32 NS token mini-groups (2 tokens each)
SCALE = DH ** -0.5
WNS_SCALE = 64.0

n_kc_s = LS // 128          # 32 S key chunks
n_kc = n_kc_s + 1           # + NS chunk
lqs = LS - QS               # 2048 S-query columns


def build_program(repeat=1):
    nc = bacc.Bacc("TRN2", target_bir_lowering=False, debug=False,
                   num_devices=N_CORES)

    xt_d = nc.dram_tensor("xt", [NT, 128, NCH, ST], BF16, kind="ExternalInput")
    xnst_d = nc.dram_tensor("xnst", [128, NCH, LNS], F8E4, kind="ExternalInput")
    ws_d = nc.dram_tensor("ws", [128, NCH, O3], BF16, kind="ExternalInput")
    wns_d = nc.dram_tensor("wns", [NG2, 128, NCH // 2, 2, 2, O3], F8E4,
                           kind="ExternalInput")
    wout_d = nc.dram_tensor("wout", [128, D], BF16, kind="ExternalInput")
    vones_d = nc.dram_tensor("vones", [128, 64], BF16, kind="ExternalInput")
    o_d = nc.dram_tensor("o", [LQ, D], BF16, kind="ExternalOutput")

    with tile.TileContext(nc) as tc:
      for _rep in range(repeat):
        import contextlib
        ctx = contextlib.ExitStack()
        with ctx:
            const = ctx.enter_context(tc.tile_pool(name="const", bufs=1))
            store = ctx.enter_context(tc.tile_pool(name="store", bufs=1))

            # --- constants ---
            # ws first: it gates the very first projection matmul. All the
            # all-ones tiles are memset on gpsimd instead of DMA'd (the
            # strided ones-scatter DMAs took ~3us each to issue on sync).
            ws_sb = const.tile([128, NCH, O3], BF16)
            nc.sync.dma_start(out=ws_sb, in_=ws_d.ap())
            ones_bf = const.tile([128, 64], BF16)
            nc.gpsimd.memset(ones_bf[:, :], 1.0)
            xnst_sb = const.tile([128, NCH, LNS], F8E4)
            nc.sync.dma_start(out=xnst_sb, in_=xnst_d.ap())
            # scaled identity (1/WNS_SCALE on the diagonal) for the NS
            # transposes - folds the W_NS descale into the PE transpose
            ident_sb = const.tile([64, 64], BF16)
            nc.gpsimd.memset(ident_sb[:, :], 0.0)
            nc.gpsimd.affine_select(
                out=ident_sb[:, :], in_=ident_sb[:, :],
                compare_op=mybir.AluOpType.not_equal, fill=1.0 / WNS_SCALE,
                base=0, channel_multiplier=1, pattern=[[-1, 64]])
            wout_sb = const.tile([128, D], BF16)

            # --- persistent activation storage (all bf16) ---
            kt_s = store.tile([128, LS], BF16)      # K^T (h0 rows 0-63, h1 64-127)
            qt_s = store.tile([128, lqs], BF16)     # Q^T, S part
            kt_ns = store.tile([128, LNS], BF16)
            qt_ns = store.tile([128, LNS], BF16)
            # V natural layout: [keys, kc, head, dh]
            v_s = store.tile([128, n_kc_s, HPC, DH], BF16)
            v_ns = store.tile([64, HPC, DH], BF16)
            qkvns_sb = store.tile([LNS, O3], BF16)  # NS qkv rows (x64 scaled)

            xpool = ctx.enter_context(tc.tile_pool(name="xpool", bufs=3))
            wnspool = ctx.enter_context(tc.tile_pool(name="wnspool", bufs=6))
            expool = ctx.enter_context(tc.tile_pool(name="expool", bufs=5))
            dapool = ctx.enter_context(tc.tile_pool(name="dapool", bufs=4))
            rcpool = ctx.enter_context(tc.tile_pool(name="rcpool", bufs=2))
            avtnpool = ctx.enter_context(tc.tile_pool(name="avtnpool", bufs=2))
            outpool = ctx.enter_context(tc.tile_pool(name="outpool", bufs=2))
            stgpool = ctx.enter_context(tc.tile_pool(name="stgpool", bufs=2))
            psS = ctx.enter_context(tc.tile_pool(name="psS", bufs=3, space="PSUM"))
            psAV = ctx.enter_context(tc.tile_pool(name="psAV", bufs=2, space="PSUM"))

            # ---------------- S-token projection units ----------------
            # Emitted lazily; tiles in dependency order [4,0,1,2,3] then
            # [5,6,7] interleaved into the attention chunk loops.
            def sproj_units():
                first = True
                for t in (4, 0, 1, 2, 3, 5, 6, 7):
                    s0 = t * ST
                    xt_t = xpool.tile([128, NCH, ST], BF16, tag="xt", name="xt_t")
                    if first:
                        # split the first tile's load so the first projection
                        # matmul (which reads ci=0 first) starts sooner
                        first = False
                        nc.sync.dma_start(out=xt_t[:, 0:2, :],
                                          in_=xt_d.ap()[t][:, 0:2, :])
                        nc.sync.dma_start(out=xt_t[:, 2:NCH, :],
                                          in_=xt_d.ap()[t][:, 2:NCH, :])
                    else:
                        nc.sync.dma_start(out=xt_t, in_=xt_d.ap()[t])
                    # K^T (and Q^T for t>=4): W_S columns stationary, x moving
                    jobs = [(1, kt_s, s0)]
                    if t >= 4:
                        jobs.append((0, qt_s, s0 - QS))
                    for mi, dest, dcol in jobs:
                        ps = psS.tile([128, 2, QT], F32, tag="psS", name="ps_kq")
                        for ci in range(NCH):
                            nc.tensor.matmul(
                                ps[:, 0, :],
                                lhsT=ws_sb[:, ci, mi * 128:(mi + 1) * 128],
                                rhs=xt_t[:, ci, :],
                                start=(ci == 0), stop=(ci == NCH - 1))
                        nc.vector.tensor_copy(out=dest[:, dcol:dcol + ST],
                                              in_=ps[:, 0, :])
                        yield
                    # V natural: x^T chunk stationary, W_S V-cols moving
                    for sub in range(ST // 128):
                        kc = 4 * t + sub
                        psv = psS.tile([128, 2, QT], F32, tag="psS", name="psv")
                        for ci in range(NCH):
                            nc.tensor.matmul(
                                psv[:, 0, 0:128],
                                lhsT=xt_t[:, ci, sub * 128:(sub + 1) * 128],
                                rhs=ws_sb[:, ci, 256:384],
                                start=(ci == 0), stop=(ci == NCH - 1))
                        # both heads in one strided CAST
                        nc.vector.tensor_copy(
                            out=v_s[:, kc, :, 0:64],
                            in_=psv[:, 0, 0:128])
                        yield

            sproj = sproj_units()
            sproj_left = 8 * 4 + 8 + 4   # 44 units total

            def sproj_step(n=1):
                nonlocal sproj_left
                for _ in range(n):
                    if sproj_left > 0:
                        next(sproj)
                        sproj_left -= 1

            # ---------------- NS-token projection mini-groups ----------------
            wns_tiles = {}
            wns_next_prefetch = 0

            def wns_prefetch():
                nonlocal wns_next_prefetch
                g = wns_next_prefetch
                if g >= NG2:
                    return
                wns_next_prefetch += 1
                wt = wnspool.tile([128, NCH // 2, 2, 2, O3], F8E4, tag="wns",
                                  name=f"wns_t{g}")
                nc.sync.dma_start(out=wt, in_=wns_d.ap()[g])
                wns_tiles[g] = wt

            def ns_units():
                # DoubleRow fp8e4: K=256 per matmul (chunk pair via the
                # [Ki, Ko=2, dim] interleave). One mini-group = 2 tokens,
                # one psS tile (token j in bank j = its own acc group).
                for g in range(NG2):
                    # keep the DMA prefetch ~4 groups ahead
                    while wns_next_prefetch < min(g + 4, NG2):
                        wns_prefetch()
                    wt = wns_tiles.pop(g)
                    ps = psS.tile([128, 2, QT], F32, tag="psS", name="ns_ps")
                    for j in range(2):
                        tok = 2 * g + j
                        for cp in range(NCH // 2):
                            nc.tensor.matmul(
                                ps[0:1, j, 0:O3],
                                lhsT=xnst_sb[:, 2 * cp:2 * cp + 2, tok:tok + 1],
                                rhs=wt[:, cp, j, :, :],
                                start=(cp == 0), stop=(cp == NCH // 2 - 1),
                                perf_mode=mybir.MatmulPerfMode.DoubleRow)
                    stg = stgpool.tile([1, 2, O3], BF16, tag="stg", name="stg")
                    nc.vector.tensor_copy(out=stg[0:1, :, :],
                                          in_=ps[0:1, 0:2, 0:O3])
                    nc.sync.dma_start(out=qkvns_sb[2 * g:2 * g + 2, :],
                                      in_=stg[0:1, :, :])
                    yield

            nsgen = ns_units()
            ns_left = NG2
            ns_finalized = [False]

            def ns_step(n=1):
                nonlocal ns_left
                for _ in range(n):
                    if ns_left > 0:
                        next(nsgen)
                        ns_left -= 1

            def ns_finalize():
                # Q_NS^T / K_NS^T via plain matmul against the 1/64-scaled
                # identity (transpose + descale in one); V_NS by 1/64 muls.
                for part, dest in ((0, qt_ns), (1, kt_ns)):
                    pst = psS.tile([128, 2, QT], F32, tag="psS", name="pst")
                    nc.tensor.matmul(
                        pst[:, 0, 0:64],
                        lhsT=qkvns_sb[0:64, part * 128:(part + 1) * 128],
                        rhs=ident_sb[:, :], start=True, stop=True)
                    nc.vector.tensor_copy(out=dest[:, :], in_=pst[:, 0, 0:64])
                for h in range(2):
                    nc.vector.tensor_scalar_mul(
                        v_ns[0:64, h, 0:64],
                        qkvns_sb[0:64, 256 + h * 64:256 + (h + 1) * 64],
                        1.0 / WNS_SCALE)

            # consume tiles 4,0,1,2,3 up-front (attention q-tile 0 deps):
            # tile4: K,Q,V0-3 = 6 units; tiles 0-3: K,V0-3 = 5 units each
            sproj_step(2)            # xt4 DMA + K4 + Q4 right behind ws
            wns_prefetch()
            wns_prefetch()
            sproj_step(4 + 4 * 5)
            # wout load after the critical-path constants
            nc.sync.dma_start(out=wout_sb, in_=wout_d.ap())

            # ---------------- main attention loop ----------------
            q_tiles = [(q0, min(QT, LQ - q0)) for q0 in range(0, LQ, QT)]
            pending_boundary = None

            for qt_i, (q0, qw) in enumerate(q_tiles):
                kc_count = min((QS + q0 + qw - 1) // 128 + 1, n_kc)
                is_last_qt = (q0 >= lqs)

                # everything q-tile qt_i reads (tiles <= 4+qt_i) must be
                # emitted before its chunk loop emits the readers
                need_done = min(26 + 6 * qt_i, 44)
                sproj_step(max(0, need_done - (44 - sproj_left)))

                # av tile allocated lazily (after the previous q-tile's
                # deferred boundary has emitted its reads of the old one).
                # Col-tiled: h0 -> psum partitions 0-63, h1 -> 64-127.
                # Softmax denominators accumulate on DVE in bf16 (even/odd
                # chunk split halves the rounding walk and deepens pipelining)
                av = [None]
                da = [None, None]
                da_used = [False, False]

                def alloc_av():
                    av[0] = psAV.tile([128, QT], F32, tag="av", name="av")
                    da[0] = dapool.tile([128, 2, QT], BF16, tag="da", name="da0")
                    da[1] = dapool.tile([128, 2, QT], BF16, tag="da", name="da1")
                    da_used[0] = da_used[1] = False

                def qk_pair(kc, ex_tiles):
                    # qlo: first unmasked query column for this key chunk
                    qlo = max(0, 128 * kc - (QS + q0)) if not is_last_qt else 0
                    is_ns_chunk = (kc == n_kc_s)
                    kw = LNS if is_ns_chunk else 128
                    ps = psS.tile([128, 2, QT], F32, tag="psS", name="ps_s")
                    for h in range(2):
                        hs = slice(h * 64, h * 64 + 64)
                        if is_ns_chunk:
                            k_src = kt_ns[hs, 0:kw]
                        else:
                            k_src = kt_s[hs, kc * 128:kc * 128 + kw]
                        if is_last_qt:
                            q_src = qt_ns[hs, q0 - lqs:q0 - lqs + qw]
                        else:
                            q_src = qt_s[hs, q0 + qlo:q0 + qw]
                        nc.tensor.matmul(ps[0:kw, h, qlo:qw], lhsT=k_src,
                                         rhs=q_src, start=True, stop=True)
                    ex = expool.tile([128, 2, QT], BF16, tag="exp", name="ex")
                    nc.scalar.activation(
                        out=ex[0:kw, :, qlo:qw], in_=ps[0:kw, :, qlo:qw],
                        func=mybir.ActivationFunctionType.Exp, scale=SCALE)
                    if 128 * kc + kw - 1 > QS + q0:
                        for h in range(2):
                            nc.gpsimd.affine_select(
                                out=ex[0:kw, h, qlo:qw], in_=ex[0:kw, h, qlo:qw],
                                compare_op=mybir.AluOpType.is_ge, fill=0.0,
                                base=QS + q0 + qlo - 128 * kc,
                                channel_multiplier=-1,
                                pattern=[[1, qw - qlo]])
                    ex_tiles[kc] = (ex, qlo)

                def dn_accum(kc, kw, ex, qlo, co=None):
                    # denominator accumulate; the even-parity chain runs on
                    # DVE, the odd-parity chain on GpSimd (both operands live
                    # in SBUF, and this splits ~70us of adds across the two
                    # engines). co is the column base in the (shared) ex tile
                    # for the NS q-tile grouped path.
                    src = ex[0:kw, :, qlo:qw] if co is None else \
                        ex[0:kw, :, co:co + qw]
                    a = kc % 2
                    eng = nc.vector
                    dst = da[a][0:kw, :, qlo:qw]
                    if not da_used[a]:
                        # first chunk of each parity covers the full q range
                        # (qlo == 0) and all 128 key partitions
                        assert qlo == 0 and kw == 128
                        da_used[a] = True
                        eng.tensor_copy(out=dst, in_=src)
                    else:
                        eng.tensor_add(dst, da[a][0:kw, :, qlo:qw], src)

                def av_pair(kc, ex_tiles, last_kc):
                    is_ns_chunk = (kc == n_kc_s)
                    kw = LNS if is_ns_chunk else 128
                    ex, qlo = ex_tiles.pop(kc)
                    for h in range(2):
                        v_src = v_ns[0:kw, h, :] if is_ns_chunk \
                            else v_s[0:kw, kc, h, :]
                        nc.tensor.matmul(av[0][64 * h:64 * h + 64, qlo:qw],
                                         lhsT=v_src,
                                         rhs=ex[0:kw, h, qlo:qw],
                                         start=(kc == 0),
                                         stop=(kc == last_kc))
                    dn_accum(kc, kw, ex, qlo)

                def ns_qtile_group(c0, c1, ex_tiles):
                    # batch chunks [c0, c1) of the 64-wide NS q-tile into one
                    # scores tile at 64-col offsets; one exp per head group
                    ps = psS.tile([128, 2, QT], F32, tag="psS", name="ps_g")
                    ex = expool.tile([128, 2, QT], BF16, tag="exp", name="exg")
                    for kc in range(c0, c1):
                        is_ns_chunk = (kc == n_kc_s)
                        kw = LNS if is_ns_chunk else 128
                        co = 64 * (kc - c0)
                        for h in range(2):
                            hs = slice(h * 64, h * 64 + 64)
                            k_src = kt_ns[hs, 0:kw] if is_ns_chunk \
                                else kt_s[hs, kc * 128:kc * 128 + kw]
                            q_src = qt_ns[hs, 0:qw]
                            nc.tensor.matmul(ps[0:kw, h, co:co + qw],
                                             lhsT=k_src, rhs=q_src,
                                             start=True, stop=True)
                    wide = 64 * (c1 - c0)
                    kwmax = 128 if c1 - 1 < n_kc_s or c1 - c0 > 1 else LNS
                    nc.scalar.activation(
                        out=ex[0:kwmax, :, 0:wide], in_=ps[0:kwmax, :, 0:wide],
                        func=mybir.ActivationFunctionType.Exp, scale=SCALE)
                    for kc in range(c0, c1):
                        if kc == n_kc_s:
                            co = 64 * (kc - c0)
                            for h in range(2):
                                nc.gpsimd.affine_select(
                                    out=ex[0:LNS, h, co:co + qw],
                                    in_=ex[0:LNS, h, co:co + qw],
                                    compare_op=mybir.AluOpType.is_ge, fill=0.0,
                                    base=0, channel_multiplier=-1,
                                    pattern=[[1, qw]])
                    for kc in range(c0, c1):
                        ex_tiles[kc] = (ex, 64 * (kc - c0))

                def ns_av(kc, ex_tiles, last_kc):
                    is_ns_chunk = (kc == n_kc_s)
                    kw = LNS if is_ns_chunk else 128
                    ex, co = ex_tiles[kc]
                    for h in range(2):
                        v_src = v_ns[0:kw, h, :] if is_ns_chunk \
                            else v_s[0:kw, kc, h, :]
                        nc.tensor.matmul(av[0][64 * h:64 * h + 64, 0:qw],
                                         lhsT=v_src,
                                         rhs=ex[0:kw, h, co:co + qw],
                                         start=(kc == 0),
                                         stop=(kc == last_kc))
                    dn_accum(kc, kw, ex, 0, co)

                def make_boundary(av, da, q0, qw):
                    # normalize + W_out for a finished q-tile; deferred so it
                    # overlaps the next q-tile's first chunks instead of
                    # stalling the tensor queue
                    def boundary():
                        # reduce the bf16 denominator accumulators over the
                        # key partitions, broadcast 64-wide per head: one
                        # psum bank, h0 -> partitions 0-63, h1 -> 64-127
                        pbc = psS.tile([128, 2, QT], F32, tag="psS", name="pbc")
                        for h in range(2):
                            for a in range(2):
                                nc.tensor.matmul(
                                    pbc[64 * h:64 * h + 64, 0, 0:qw],
                                    lhsT=ones_bf[:, :],
                                    rhs=da[a][:, h, 0:qw],
                                    start=(a == 0), stop=(a == 1))
                        rcf = rcpool.tile([128, QT], F32, tag="rcf", name="rcf")
                        nc.vector.reciprocal_approx_fast(
                            out=rcf[:, 0:qw], in_=pbc[:, 0, 0:qw])
                        avtn = avtnpool.tile([128, QT], BF16, tag="avtn",
                                             name="avtn")
                        with nc.allow_low_precision(
                                reason="bf16 softmax normalize"):
                            nc.vector.tensor_mul(avtn[:, 0:qw], av[0][:, 0:qw],
                                                 rcf[:, 0:qw])
                        for qs in range(math.ceil(qw / 128)):
                            qsw = min(128, qw - qs * 128)
                            po = psS.tile([128, 2, QT], F32, tag="psS", name="po")
                            ot = outpool.tile([128, D], BF16, tag="out", name="ot")
                            for e in range(2):
                                nc.tensor.matmul(
                                    po[0:qsw, e, :],
                                    lhsT=avtn[:, qs * 128:qs * 128 + qsw],
                                    rhs=wout_sb[:, e * 512:(e + 1) * 512],
                                    start=True, stop=True)
                                # split the psum->sbuf output casts across
                                # ScalarE and DVE so they run in parallel
                                if e == 0:
                                    nc.scalar.copy(
                                        out=ot[0:qsw, e * 512:(e + 1) * 512],
                                        in_=po[0:qsw, e, :])
                                else:
                                    nc.vector.tensor_copy(
                                        out=ot[0:qsw, e * 512:(e + 1) * 512],
                                        in_=po[0:qsw, e, :])
                            nc.sync.dma_start(
                                out=o_d.ap()[q0 + qs * 128:q0 + qs * 128 + qsw, :],
                                in_=ot[0:qsw, :])
                    return boundary

                ex_tiles = {}
                if is_last_qt:
                    # drain any remaining NS projection + finalize first
                    ns_step(ns_left)
                    sproj_step(sproj_left)
                    if not ns_finalized[0]:
                        ns_finalized[0] = True
                        ns_finalize()
                    if pending_boundary is not None:
                        pending_boundary()
                        pending_boundary = None
                    alloc_av()
                    groups = [(c0, min(c0 + 8, kc_count))
                              for c0 in range(0, kc_count, 8)]
                    for gi, (c0, c1) in enumerate(groups):
                        ns_qtile_group(c0, c1, ex_tiles)
                        if gi > 0:
                            p0, p1 = groups[gi - 1]
                            for kc in range(p0, p1):
                                ns_av(kc, ex_tiles, kc_count - 1)
                    p0, p1 = groups[-1]
                    for kc in range(p0, p1):
                        ns_av(kc, ex_tiles, kc_count - 1)
                    ex_tiles.clear()
                    make_boundary(av, da, q0, qw)()
                else:
                    # two-chunk super-steps: QK for two chunks back-to-back,
                    # then the lagged AVs back-to-back - fewer stationary-
                    # operand transitions on the PE
                    av_done = [0]
                    for kc in range(kc_count):
                        qk_pair(kc, ex_tiles)
                        if kc == 1:
                            alloc_av()
                        if kc == 3 and pending_boundary is not None:
                            # deferred 3 chunks in: by now the previous
                            # q-tile's DVE normalize chain has drained, so
                            # these tensor-queue ops won't head-of-line block
                            pending_boundary()
                            pending_boundary = None
                        if kc >= 3 and kc % 2 == 1:
                            while av_done[0] < kc:
                                av_pair(av_done[0], ex_tiles, kc_count - 1)
                                av_done[0] += 1
                        # interleave fillers: NS mini-groups and the
                        # remaining s-proj tiles, at most one per chunk
                        if kc % 3 == 2:
                            ns_step()
                        elif kc % 3 == 0 and kc > 0:
                            sproj_step()
                        if ns_left == 0 and sproj_left == 0 \
                                and not ns_finalized[0]:
                            ns_finalized[0] = True
                            ns_finalize()
                    while av_done[0] < kc_count:
                        av_pair(av_done[0], ex_tiles, kc_count - 1)
                        av_done[0] += 1
                    pending_boundary = make_boundary(av, da, q0, qw)

    nc.compile()
    return nc


_NC_CACHE = {}


def _get_program():
    if "nc" not in _NC_CACHE:
        _NC_CACHE["nc"] = build_program()
    return _NC_CACHE["nc"]


def _prep_core(c, xt, xnst, W_S, W_NS, W_out):
    """Host-side shard prep for core c (heads 2c, 2c+1)."""
    h0 = 2 * c * DH
    cols = np.r_[h0:h0 + HPC * DH,
                 D + h0:D + h0 + HPC * DH,
                 2 * D + h0:2 * D + h0 + HPC * DH]
    ws = W_S[:, cols].reshape(NCH, 128, O3).transpose(1, 0, 2)
    ws = np.ascontiguousarray(ws.astype(ml_dtypes.bfloat16))
    # wns: [LNS, D, O3cols] -> [NG2, 128, NCH//2, 2tok, 2ko, O3] fp8 (x64)
    wns = W_NS[:, :, cols] * WNS_SCALE
    # [tok, d, o] with d = 256*cp + 128*ko + p -> [g, p, cp, tok2, ko, o]
    wns = wns.reshape(NG2, 2, NCH // 2, 2, 128, O3).transpose(0, 4, 2, 1, 3, 5)
    wns = np.ascontiguousarray(wns.astype(ml_dtypes.float8_e4m3))
    wout = np.ascontiguousarray(
        W_out[h0:h0 + HPC * DH].astype(ml_dtypes.bfloat16))
    return {"xt": xt, "xnst": xnst, "ws": ws, "wns": wns, "wout": wout,
            "vones": np.ones((128, 64), dtype=ml_dtypes.bfloat16)}


def prep_in_maps(np_inputs):
    """Host-side shard prep for all cores (used by kernel() and test timing)."""
    x = np.asarray(np_inputs["x"], dtype=np.float32)
    W_S = np.asarray(np_inputs["W_S"], dtype=np.float32)
    W_NS = np.asarray(np_inputs["W_NS"], dtype=np.float32)
    W_out = np.asarray(np_inputs["W_out"], dtype=np.float32)
    xs = x[0]
    xt = xs[:LS].T.reshape(NCH, 128, NT, ST).transpose(2, 1, 0, 3)
    xt = np.ascontiguousarray(xt.astype(ml_dtypes.bfloat16))
    xnst = xs[LS:].T.reshape(NCH, 128, LNS).transpose(1, 0, 2)
    xnst = np.ascontiguousarray(xnst.astype(ml_dtypes.float8_e4m3))
    with ThreadPoolExecutor(max_workers=N_CORES) as ex:
        return list(ex.map(
            lambda c: _prep_core(c, xt, xnst, W_S, W_NS, W_out),
            range(N_CORES)))


def kernel(x, W_S, W_NS, W_out, L_S=None, query_start=None, **_unused):
    x = np.asarray(x, dtype=np.float32)
    W_S = np.asarray(W_S, dtype=np.float32)
    W_NS = np.asarray(W_NS, dtype=np.float32)
    W_out = np.asarray(W_out, dtype=np.float32)
    if L_S is not None:
        assert int(L_S) == LS, f"kernel hardcodes L_S={LS}, got {int(L_S)}"
    if query_start is not None:
        assert int(query_start) == QS, \
            f"kernel hardcodes query_start={QS}, got {int(query_start)}"
    assert x.shape == (1, LS + LNS, D)

    nc = _get_program()
    in_maps = prep_in_maps({"x": x, "W_S": W_S, "W_NS": W_NS, "W_out": W_out})

    res = None
    for attempt in range(3):
        try:
            res = run_bass_kernel_spmd(nc, in_maps, list(range(N_CORES)))
            break
        except Exception:
            if attempt == 2:
                raise
            import time
            time.sleep(100)
    out = np.zeros((LQ, D), dtype=np.float32)
    for r in res.results:
        out += np.asarray(r["o"], dtype=np.float32)
    return out.reshape(1, LQ, D)


if __name__ == "__main__":
    rng = np.random.default_rng(0)
    ins = {
        "x": rng.standard_normal((1, LS + LNS, D), dtype=np.float32),
        "W_S": rng.standard_normal((D, 3 * D), dtype=np.float32) * 0.02,
        "W_NS": rng.standard_normal((LNS, D, 3 * D), dtype=np.float32) * 0.02,
        "W_out": rng.standard_normal((D, D), dtype=np.float32) * 0.03,
        "L_S": LS, "query_start": QS,
    }
    out = kernel(**ins)
    print("kernel out shape:", out.shape, "finite:", np.isfinite(out).all())
